# revision 19
# baseline (speedup 1.0000x reference)
"""Causal multi-head attention (B=2, T=2048, D=1024, H=16) on 8 Trainium2 cores.

Sharding: Megatron-style tensor parallelism over heads (2 heads/core) for the
QKV projections and attention; AllToAll redistributes context from head-sharded
to token-sharded so each core computes the output projection for its own 512
token slice.  Host-side work is only dtype casting, weight slicing, and
concatenating the 8 disjoint output slices.

Dataflow (all on-chip, per core):
  X^T tiles via 2-byte DMA transpose of bf16 X
  K^T,Q^T = W^T X^T  (features on partitions, tokens on free axis)
  V       = X W_v    (tokens on partitions) with a ones column appended
  S^T[s,o] = k_o . q_s  -> P = exp(S/8) * causal_mask  (softmax axis s = partitions)
  ctx^T/denom from one matmul chain: lhsT=[V | 1], rhs=P^T  (denominator free)
  normalize, AllToAll, out-projection + bias -> y[t_slice, D]
"""

import numpy as np
import ml_dtypes

B, T, D = 2, 2048, 1024
H, DH = 16, 64
NCORES = 8
HPC = H // NCORES          # heads per core = 2
BT = B * T                 # 4096
TLOC = BT // NCORES        # 512 tokens per core for out-projection
SCALE = 1.0 / 8.0          # 1/sqrt(DH)

_BF = ml_dtypes.bfloat16


def _build_nc(loop_n=1, cc=True):
    from contextlib import ExitStack

    import concourse.bass as bass
    import concourse.mybir as mybir
    import concourse.tile as tile
    from concourse import bacc

    f32 = mybir.dt.float32
    bf16 = mybir.dt.bfloat16
    Exp = mybir.ActivationFunctionType.Exp

    nc = bacc.Bacc()

    x_bf = nc.dram_tensor("x_bf", [BT, D], bf16, kind="ExternalInput")
    wk = nc.dram_tensor("wk_loc", [D, HPC * DH], bf16, kind="ExternalInput")
    wq = nc.dram_tensor("wq_loc", [D, HPC * DH], bf16, kind="ExternalInput")
    wv = nc.dram_tensor("wv_loc", [D, HPC * DH], bf16, kind="ExternalInput")
    wo = nc.dram_tensor("w_out", [D, D], bf16, kind="ExternalInput")
    bo = nc.dram_tensor("b_out", [D], f32, kind="ExternalInput")
    msk = nc.dram_tensor("masks", [4, 128, 512], bf16, kind="ExternalInput")
    y_out = nc.dram_tensor("y_out", [TLOC, D], f32, kind="ExternalOutput")

    NB = T // 128   # 16 s-tiles per batch
    NS = T // 512   # 4 o-strips per batch
    ND = D // 128   # 8 d-tiles

    with tile.TileContext(nc) as tc, ExitStack() as ctx:
        const = ctx.enter_context(tc.tile_pool(name="const", bufs=1))
        xpool = ctx.enter_context(tc.tile_pool(name="xpool", bufs=16))
        ppool = ctx.enter_context(tc.tile_pool(name="ppool", bufs=4))
        npool = ctx.enter_context(tc.tile_pool(name="npool", bufs=4))
        opool = ctx.enter_context(tc.tile_pool(name="opool", bufs=2))
        psum = ctx.enter_context(tc.tile_pool(name="psum", space="PSUM", bufs=1))
        dram = ctx.enter_context(tc.tile_pool(name="dram", space="DRAM", bufs=1))

        # ---- constants into SBUF (W_out/bias loaded late, see _emit_main) ----
        wk_sb = const.tile([128, ND, 128], bf16)
        nc.sync.dma_start(out=wk_sb, in_=wk[:, :].rearrange("(a p) f -> p a f", p=128))
        wq_sb = const.tile([128, ND, 128], bf16)
        nc.sync.dma_start(out=wq_sb, in_=wq[:, :].rearrange("(a p) f -> p a f", p=128))
        wv_sb = const.tile([128, ND, 128], bf16)
        nc.sync.dma_start(out=wv_sb, in_=wv[:, :].rearrange("(a p) f -> p a f", p=128))
        masks_sb = const.tile([128, 128], bf16)
        nc.sync.dma_start(out=masks_sb, in_=msk[0, :, 0:128])
        wo_sb = const.tile([128, ND, D], bf16)
        bo_bc = const.tile([128, D], f32)

        # ---- persistent activations ----
        kT_sb = const.tile([128, BT], bf16)          # [2*DH features, tokens]
        qT_sb = const.tile([128, BT], bf16)
        v_sb = [const.tile([128, BT // 128, DH + 1], bf16, name=f"v_sb{h}")
                for h in range(HPC)]
        for h in range(HPC):
            nc.vector.memset(v_sb[h][:, :, DH:DH + 1], 1.0)

        x_ap = x_bf[:, :]

        # ---- all-to-all buffers ----
        a2a_in = dram.tile([NCORES, 128, TLOC], bf16)
        a2a_out = dram.tile([NCORES, 128, TLOC], bf16)

        def emit_body():
            _emit_main(nc, bass, mybir, tc, ctx, Exp, f32, bf16,
                       x_ap, wk_sb, wq_sb, wv_sb, wo_sb, masks_sb, bo_bc,
                       kT_sb, qT_sb, v_sb, a2a_in, a2a_out,
                       xpool, ppool, npool, opool, psum, y_out, cc, wo, bo)

        if loop_n == 1:
            emit_body()
        else:
            hint = (mybir.EngineType.PE, mybir.EngineType.Activation,
                    mybir.EngineType.DVE, mybir.EngineType.Pool,
                    mybir.EngineType.SP)
            with tc.For_i(0, loop_n, 1, hint_engines=hint):
                emit_body()

    nc.compile()
    return nc


def _emit_main(nc, bass, mybir, tc, ctx, Exp, f32, bf16,
               x_ap, wk_sb, wq_sb, wv_sb, wo_sb, masks_sb, bo_bc,
               kT_sb, qT_sb, v_sb, a2a_in, a2a_out,
               xpool, ppool, npool, opool, psum, y_out, cc=True,
               wo=None, bo=None):
    NB = T // 128
    NS = T // 512
    ND = D // 128

    def emit_proj_chunk(n):
        # QKV projections for tokens [512n, 512n+512)
        xts = []
        for dt in range(ND):
            xt = xpool.tile([128, 512], bf16, tag="xt", name=f"xt_{n}_{dt}")
            nc.sync.dma_start(
                out=xt,
                in_=x_ap[n * 512:(n + 1) * 512, dt * 128:(dt + 1) * 128],
                transpose=True,
            )
            xts.append(xt)
        for wi, (wsb, dstT) in enumerate(((wk_sb, kT_sb), (wq_sb, qT_sb))):
            ps = psum.tile([128, 512], f32, tag="w512", bufs=2,
                           name=f"pskq_{n}_{wi}")
            for dt in range(ND):
                nc.tensor.matmul(ps, wsb[:, dt, :], xts[dt],
                                 start=(dt == 0), stop=(dt == ND - 1))
            nc.vector.tensor_copy(out=dstT[:, n * 512:(n + 1) * 512], in_=ps)
        for m in range(4):
            psv = psum.tile([128, 128], f32, tag="w512", bufs=2,
                            name=f"psv_{n}_{m}")
            for dt in range(ND):
                nc.tensor.matmul(psv, xts[dt][:, m * 128:(m + 1) * 128],
                                 wv_sb[:, dt, :],
                                 start=(dt == 0), stop=(dt == ND - 1))
            ti = n * 4 + m
            for h in range(HPC):
                nc.vector.tensor_copy(out=v_sb[h][:, ti, 0:DH],
                                      in_=psv[:, h * DH:(h + 1) * DH])

    def emit_attn_strip(b, n):
        # o-strip n of batch b, both heads interleaved; causal column skip
        o0 = T * b + 512 * n
        njt = 4 * (n + 1)
        psc = [psum.tile([DH + 1, 512], f32, tag="ctx", bufs=2,
                         name=f"psc_{b}_{n}_{h}") for h in range(HPC)]
        for jt in range(njt):
            s0 = T * b + 128 * jt
            c0 = 128 * (jt - 4 * n) if jt >= 4 * n else 0  # fully-masked cols
            pss = psum.tile([128, HPC, 512], f32, tag="big", bufs=2,
                            name=f"pss_{b}_{n}_{jt}")
            for h in range(HPC):
                hs = slice(DH * h, DH * (h + 1))
                nc.tensor.matmul(pss[:, h, c0:512], qT_sb[hs, s0:s0 + 128],
                                 kT_sb[hs, o0 + c0:o0 + 512],
                                 start=True, stop=True)
            pt = ppool.tile([128, HPC, 512], bf16, tag="pt",
                            name=f"pt_{b}_{n}_{jt}")
            nc.scalar.activation(out=pt[:, :, c0:512], in_=pss[:, :, c0:512],
                                 func=Exp, scale=SCALE)
            if jt >= 4 * n:
                for h in range(HPC):
                    nc.gpsimd.tensor_mul(pt[:, h, c0:c0 + 128],
                                         pt[:, h, c0:c0 + 128], masks_sb)
            for h in range(HPC):
                nc.tensor.matmul(psc[h][:, c0:512], v_sb[h][:, NB * b + jt, :],
                                 pt[:, h, c0:512],
                                 start=(jt == 0), stop=(jt == njt - 1))
        for h in range(HPC):
            rec = npool.tile([1, 512], f32, tag="rec", name=f"rec_{b}_{n}_{h}")
            nc.vector.reciprocal(rec, psc[h][DH:DH + 1, :])
            bc = npool.tile([DH, 512], f32, tag="bc", name=f"bc_{b}_{n}_{h}")
            nc.gpsimd.partition_broadcast(bc, rec)
            cn = npool.tile([DH, 512], bf16, tag="cn", name=f"cn_{b}_{n}_{h}")
            nc.vector.tensor_mul(cn, psc[h][0:DH, :], bc)
            nc.sync.dma_start(out=a2a_in[NS * b + n, DH * h:DH * (h + 1), :],
                              in_=cn)

    if True:
        # ---- pipelined: attention strip (b,n) right after its last chunk ----
        for c in range(BT // 512):
            emit_proj_chunk(c)
            emit_attn_strip(c // NS, c % NS)

        # ---- late constants (W_out, bias) — loaded during attention ----
        nc.sync.dma_start(out=wo_sb, in_=wo[:, :].rearrange("(a p) d -> p a d", p=128))
        bo_ap = bo[:]
        nc.gpsimd.dma_start(
            out=bo_bc,
            in_=bass.AP(tensor=bo_ap.tensor, offset=bo_ap.offset,
                        ap=[[0, 128]] + list(bo_ap.ap)),
        )

        # ---- exchange: head-sharded ctx -> token-sharded full ctx ----
        if cc:
            nc.gpsimd.collective_compute(
                "AllToAll",
                mybir.AluOpType.bypass,
                replica_groups=[list(range(NCORES))],
                ins=[a2a_in[:, :, :]],
                outs=[a2a_out[:, :, :]],
            )
        else:
            nc.sync.dma_start(out=a2a_out[:, :, :], in_=a2a_in[:, :, :])

        # ---- output projection for my 512-token slice ----
        ctxf_sb = opool.tile([128, NCORES, TLOC], bf16, bufs=1)
        nc.sync.dma_start(out=ctxf_sb, in_=a2a_out[:, :, :].rearrange("s p t -> p s t"))
        for m in range(TLOC // 128):
            y_sb = opool.tile([128, D], f32, tag="ysb", name=f"ysb_{m}")
            for st in range(D // 512):
                psy = psum.tile([128, 512], f32, tag="w512", bufs=2,
                                name=f"psy_{m}_{st}")
                for ft in range(NCORES):
                    nc.tensor.matmul(psy, ctxf_sb[:, ft, m * 128:(m + 1) * 128],
                                     wo_sb[:, ft, st * 512:(st + 1) * 512],
                                     start=(ft == 0), stop=(ft == NCORES - 1))
                nc.vector.tensor_add(y_sb[:, st * 512:(st + 1) * 512], psy,
                                     bo_bc[:, st * 512:(st + 1) * 512])
            nc.sync.dma_start(out=y_out[m * 128:(m + 1) * 128, :], in_=y_sb)


def _make_masks():
    # masks[r][p, f] = 1 where token (128*r + p) may be attended by token f
    p = np.arange(128)[:, None]
    f = np.arange(512)[None, :]
    return np.stack([(128 * r + p <= f) for r in range(4)]).astype(_BF)


_NC_CACHE = {}
LAST_RESULT = None


def _make_in_maps(input_emb, W_key, W_query, W_value, W_out, b_out):
    x = np.asarray(input_emb, dtype=np.float32).reshape(BT, D)
    wkf = np.asarray(W_key, dtype=np.float32)
    wqf = np.asarray(W_query, dtype=np.float32)
    wvf = np.asarray(W_value, dtype=np.float32)
    wof = np.asarray(W_out, dtype=np.float32)
    bof = np.ascontiguousarray(np.asarray(b_out, dtype=np.float32))

    x_bf = np.ascontiguousarray(x.astype(_BF))
    wo_bf = np.ascontiguousarray(wof.astype(_BF))
    masks = np.ascontiguousarray(_make_masks())

    in_maps = []
    for c in range(NCORES):
        sl = slice(HPC * DH * c, HPC * DH * (c + 1))
        in_maps.append({
            "x_bf": x_bf,
            "wk_loc": np.ascontiguousarray(wkf[:, sl].astype(_BF)),
            "wq_loc": np.ascontiguousarray(wqf[:, sl].astype(_BF)),
            "wv_loc": np.ascontiguousarray(wvf[:, sl].astype(_BF)),
            "w_out": wo_bf,
            "b_out": bof,
            "masks": masks,
        })
    return in_maps


def bench(inputs, n_iters=16, loop_n=1, cc=True):
    """Time the NEFF execution with device-resident inputs.

    Returns (per_call_seconds, y) where y is from the last iteration.
    """
    import time

    import jax
    from jax.sharding import Mesh, NamedSharding, PartitionSpec
    from jax.experimental.shard_map import shard_map

    import concourse.mybir as mybir
    from concourse import bass2jax

    bass2jax.install_neuronx_cc_hook()

    if ("nc", loop_n, cc) not in _NC_CACHE:
        _NC_CACHE[("nc", loop_n, cc)] = _build_nc(loop_n, cc)
    nc = _NC_CACHE[("nc", loop_n, cc)]
    in_maps = _make_in_maps(**inputs)

    partition_name = nc.partition_id_tensor.name if nc.partition_id_tensor else None
    in_names, out_names, out_avals, zero_outs = [], [], [], []
    for alloc in nc.m.functions[0].allocations:
        if not isinstance(alloc, mybir.MemoryLocationSet):
            continue
        name = alloc.memorylocations[0].name
        if alloc.kind == "ExternalInput":
            if name != partition_name:
                in_names.append(name)
        elif alloc.kind == "ExternalOutput":
            out_names.append(name)
            shape = tuple(alloc.tensor_shape)
            dtype = mybir.dt.np(alloc.dtype)
            out_avals.append(jax.core.ShapedArray(shape, dtype))
            zero_outs.append(np.zeros(shape, dtype))
    n_params = len(in_names)
    n_outs = len(out_avals)
    all_in_names = list(in_names) + list(out_names)
    if partition_name is not None:
        all_in_names.append(partition_name)
    donate = tuple(range(n_params, n_params + n_outs))

    def _body(*args):
        operands = list(args)
        if partition_name is not None:
            operands.append(bass2jax.partition_id_tensor())
        outs = bass2jax._bass_exec_p.bind(
            *operands,
            out_avals=tuple(out_avals),
            in_names=tuple(all_in_names),
            out_names=tuple(out_names),
            lowering_input_output_aliases=(),
            sim_require_finite=True,
            sim_require_nnan=True,
            nc=nc,
        )
        return tuple(outs)

    devices = jax.devices()[:NCORES]
    mesh = Mesh(np.asarray(devices), ("core",))
    in_specs = (PartitionSpec("core"),) * (n_params + n_outs)
    out_specs = (PartitionSpec("core"),) * len(out_names)
    fn = jax.jit(
        shard_map(_body, mesh=mesh, in_specs=in_specs, out_specs=out_specs,
                  check_rep=False),
        donate_argnums=donate, keep_unused=True,
    )
    sh = NamedSharding(mesh, PartitionSpec("core"))
    concat_in = [
        jax.device_put(
            np.concatenate([np.asarray(in_maps[c][nm]) for c in range(NCORES)],
                           axis=0), sh)
        for nm in in_names
    ]
    zero_sets = [
        [jax.device_put(np.zeros((NCORES * z.shape[0], *z.shape[1:]), z.dtype), sh)
         for z in zero_outs]
        for _ in range(n_iters + 1)
    ]
    out = fn(*concat_in, *zero_sets[0])
    jax.block_until_ready(out)
    t0 = time.perf_counter()
    outs = [fn(*concat_in, *zs) for zs in zero_sets[1:]]
    jax.block_until_ready(outs)
    t1 = time.perf_counter()
    per_call = (t1 - t0) / n_iters

    y = np.asarray(outs[-1][out_names.index("y_out")])
    y = y.reshape(NCORES, TLOC, D).reshape(BT, D)
    return per_call, np.ascontiguousarray(y.reshape(B, T, D).astype(np.float32))


def kernel(input_emb, W_key, W_query, W_value, W_out, b_out, _trace=False):
    from concourse.bass_utils import run_bass_kernel_spmd

    in_maps = _make_in_maps(input_emb, W_key, W_query, W_value, W_out, b_out)

    if ("nc", 1) not in _NC_CACHE:
        _NC_CACHE[("nc", 1)] = _build_nc(1)
    nc = _NC_CACHE[("nc", 1)]

    global LAST_RESULT
    LAST_RESULT = run_bass_kernel_spmd(nc, in_maps, core_ids=list(range(NCORES)),
                                       trace=_trace)
    y = np.concatenate([LAST_RESULT.results[c]["y_out"] for c in range(NCORES)],
                       axis=0)
    return np.ascontiguousarray(y.reshape(B, T, D).astype(np.float32))


def bench_hw(inputs, n1=1, n2=9, n_iters=12):
    """Two-point device-loop timing (collective replaced by local copy):
    returns (compute seconds/iter, cc-vs-copy delta seconds, y_from_n1)."""
    t_a, y = bench(inputs, n_iters=n_iters, loop_n=n1, cc=False)
    t_b, _ = bench(inputs, n_iters=n_iters, loop_n=n2, cc=False)
    t_cc, _ = bench(inputs, n_iters=max(n_iters, 24), loop_n=1, cc=True)
    t_nocc, _ = bench(inputs, n_iters=max(n_iters, 24), loop_n=1, cc=False)
    return (t_b - t_a) / (n2 - n1), t_cc - t_nocc, y


# revision 21
# speedup vs baseline: 3.6758x; 3.6758x over previous
"""Causal multi-head attention (B=2, T=2048, D=1024, H=16) on 8 Trainium2 cores.

Sharding: Megatron-style tensor parallelism over heads (2 heads/core) for the
QKV projections and attention; AllToAll redistributes context from head-sharded
to token-sharded so each core computes the output projection for its own 512
token slice.  Host-side work is only dtype casting, weight slicing, and
concatenating the 8 disjoint output slices.

Dataflow (all on-chip, per core):
  X^T tiles via 2-byte DMA transpose of bf16 X
  K^T,Q^T = W^T X^T  (features on partitions, tokens on free axis)
  V       = X W_v    (tokens on partitions) with a ones column appended
  S^T[s,o] = k_o . q_s  -> P = exp(S/8) * causal_mask  (softmax axis s = partitions)
  ctx^T/denom from one matmul chain: lhsT=[V | 1], rhs=P^T  (denominator free)
  normalize, AllToAll, out-projection + bias -> y[t_slice, D]
"""

import numpy as np
import ml_dtypes

B, T, D = 2, 2048, 1024
H, DH = 16, 64
NCORES = 8
HPC = H // NCORES          # heads per core = 2
BT = B * T                 # 4096
TLOC = BT // NCORES        # 512 tokens per core for out-projection
SCALE = 1.0 / 8.0          # 1/sqrt(DH)

_BF = ml_dtypes.bfloat16


def _build_nc(loop_n=1, cc=True):
    from contextlib import ExitStack

    import concourse.bass as bass
    import concourse.mybir as mybir
    import concourse.tile as tile
    from concourse import bacc

    f32 = mybir.dt.float32
    bf16 = mybir.dt.bfloat16
    Exp = mybir.ActivationFunctionType.Exp

    nc = bacc.Bacc()

    x_bf = nc.dram_tensor("x_bf", [BT, D], bf16, kind="ExternalInput")
    wk = nc.dram_tensor("wk_loc", [D, HPC * DH], bf16, kind="ExternalInput")
    wq = nc.dram_tensor("wq_loc", [D, HPC * DH], bf16, kind="ExternalInput")
    wv = nc.dram_tensor("wv_loc", [D, HPC * DH], bf16, kind="ExternalInput")
    wo = nc.dram_tensor("w_out", [D, D], bf16, kind="ExternalInput")
    bo = nc.dram_tensor("b_out", [D], f32, kind="ExternalInput")
    msk = nc.dram_tensor("masks", [4, 128, 512], bf16, kind="ExternalInput")
    y_out = nc.dram_tensor("y_out", [TLOC, D], f32, kind="ExternalOutput")

    NB = T // 128   # 16 s-tiles per batch
    NS = T // 512   # 4 o-strips per batch
    ND = D // 128   # 8 d-tiles

    with tile.TileContext(nc) as tc, ExitStack() as ctx:
        const = ctx.enter_context(tc.tile_pool(name="const", bufs=1))
        xpool = ctx.enter_context(tc.tile_pool(name="xpool", bufs=16))
        ppool = ctx.enter_context(tc.tile_pool(name="ppool", bufs=4))
        npool = ctx.enter_context(tc.tile_pool(name="npool", bufs=4))
        opool = ctx.enter_context(tc.tile_pool(name="opool", bufs=2))
        psum = ctx.enter_context(tc.tile_pool(name="psum", space="PSUM", bufs=1))
        dram = ctx.enter_context(tc.tile_pool(name="dram", space="DRAM", bufs=1))

        # ---- constants into SBUF (W_out/bias loaded late, see _emit_main) ----
        wk_sb = const.tile([128, ND, 128], bf16)
        nc.sync.dma_start(out=wk_sb, in_=wk[:, :].rearrange("(a p) f -> p a f", p=128))
        wq_sb = const.tile([128, ND, 128], bf16)
        nc.sync.dma_start(out=wq_sb, in_=wq[:, :].rearrange("(a p) f -> p a f", p=128))
        wv_sb = const.tile([128, ND, 128], bf16)
        nc.sync.dma_start(out=wv_sb, in_=wv[:, :].rearrange("(a p) f -> p a f", p=128))
        masks_sb = const.tile([128, 128], bf16)
        nc.sync.dma_start(out=masks_sb, in_=msk[0, :, 0:128])
        wo_sb = const.tile([128, ND, D], bf16)
        bo_bc = const.tile([128, D], f32)

        # ---- persistent activations ----
        kT_sb = const.tile([128, BT], bf16)          # [2*DH features, tokens]
        qT_sb = const.tile([128, BT], bf16)
        v_sb = [const.tile([128, BT // 128, DH + 1], bf16, name=f"v_sb{h}")
                for h in range(HPC)]
        for h in range(HPC):
            nc.vector.memset(v_sb[h][:, :, DH:DH + 1], 1.0)

        x_ap = x_bf[:, :]

        # ---- all-to-all buffers ----
        a2a_in = dram.tile([NCORES, 128, TLOC], bf16)
        a2a_out = dram.tile([NCORES, 128, TLOC], bf16)

        def emit_body():
            _emit_main(nc, bass, mybir, tc, ctx, Exp, f32, bf16,
                       x_ap, wk_sb, wq_sb, wv_sb, wo_sb, masks_sb, bo_bc,
                       kT_sb, qT_sb, v_sb, a2a_in, a2a_out,
                       xpool, ppool, npool, opool, psum, y_out, cc, wo, bo)

        if loop_n == 1:
            emit_body()
        else:
            hint = (mybir.EngineType.PE, mybir.EngineType.Activation,
                    mybir.EngineType.DVE, mybir.EngineType.Pool,
                    mybir.EngineType.SP)
            with tc.For_i(0, loop_n, 1, hint_engines=hint):
                emit_body()

    nc.compile()
    return nc


def _emit_main(nc, bass, mybir, tc, ctx, Exp, f32, bf16,
               x_ap, wk_sb, wq_sb, wv_sb, wo_sb, masks_sb, bo_bc,
               kT_sb, qT_sb, v_sb, a2a_in, a2a_out,
               xpool, ppool, npool, opool, psum, y_out, cc=True,
               wo=None, bo=None):
    NB = T // 128
    NS = T // 512
    ND = D // 128

    def emit_proj_chunk(n):
        # QKV projections for tokens [512n, 512n+512)
        xts = []
        for dt in range(ND):
            xt = xpool.tile([128, 512], bf16, tag="xt", name=f"xt_{n}_{dt}")
            nc.sync.dma_start(
                out=xt,
                in_=x_ap[n * 512:(n + 1) * 512, dt * 128:(dt + 1) * 128],
                transpose=True,
            )
            xts.append(xt)
        for wi, (wsb, dstT) in enumerate(((wk_sb, kT_sb), (wq_sb, qT_sb))):
            ps = psum.tile([128, 512], f32, tag="w512", bufs=2,
                           name=f"pskq_{n}_{wi}")
            for dt in range(ND):
                nc.tensor.matmul(ps, wsb[:, dt, :], xts[dt],
                                 start=(dt == 0), stop=(dt == ND - 1))
            nc.vector.tensor_copy(out=dstT[:, n * 512:(n + 1) * 512], in_=ps)
        for m in range(4):
            psv = psum.tile([128, 128], f32, tag="w512", bufs=2,
                            name=f"psv_{n}_{m}")
            for dt in range(ND):
                nc.tensor.matmul(psv, xts[dt][:, m * 128:(m + 1) * 128],
                                 wv_sb[:, dt, :],
                                 start=(dt == 0), stop=(dt == ND - 1))
            ti = n * 4 + m
            for h in range(HPC):
                nc.vector.tensor_copy(out=v_sb[h][:, ti, 0:DH],
                                      in_=psv[:, h * DH:(h + 1) * DH])

    def emit_attn_strip(b, n):
        # o-strip n of batch b, both heads interleaved; causal column skip
        o0 = T * b + 512 * n
        njt = 4 * (n + 1)
        psc = [psum.tile([DH + 1, 512], f32, tag="ctx", bufs=2,
                         name=f"psc_{b}_{n}_{h}") for h in range(HPC)]
        for jt in range(njt):
            s0 = T * b + 128 * jt
            c0 = 128 * (jt - 4 * n) if jt >= 4 * n else 0  # fully-masked cols
            pss = psum.tile([128, HPC, 512], f32, tag="big", bufs=2,
                            name=f"pss_{b}_{n}_{jt}")
            for h in range(HPC):
                hs = slice(DH * h, DH * (h + 1))
                nc.tensor.matmul(pss[:, h, c0:512], qT_sb[hs, s0:s0 + 128],
                                 kT_sb[hs, o0 + c0:o0 + 512],
                                 start=True, stop=True)
            pt = ppool.tile([128, HPC, 512], bf16, tag="pt",
                            name=f"pt_{b}_{n}_{jt}")
            nc.scalar.activation(out=pt[:, :, c0:512], in_=pss[:, :, c0:512],
                                 func=Exp, scale=SCALE)
            if jt >= 4 * n:
                for h in range(HPC):
                    nc.gpsimd.tensor_mul(pt[:, h, c0:c0 + 128],
                                         pt[:, h, c0:c0 + 128], masks_sb)
            for h in range(HPC):
                nc.tensor.matmul(psc[h][:, c0:512], v_sb[h][:, NB * b + jt, :],
                                 pt[:, h, c0:512],
                                 start=(jt == 0), stop=(jt == njt - 1))
        for h in range(HPC):
            dsb = npool.tile([1, 512], f32, tag="dsb", name=f"dsb_{b}_{n}_{h}")
            nc.vector.tensor_copy(out=dsb, in_=psc[h][DH:DH + 1, :])
            dbc = npool.tile([DH, 512], f32, tag="dbc", name=f"dbc_{b}_{n}_{h}")
            nc.gpsimd.partition_broadcast(dbc, dsb)
            bc = npool.tile([DH, 512], f32, tag="bc", name=f"bc_{b}_{n}_{h}")
            rsc = npool.tile([DH, 512], f32, tag="rsc", name=f"rsc_{b}_{n}_{h}")
            nc.vector.reciprocal_approx_accurate(bc, dbc, rsc)
            cn = npool.tile([DH, 512], bf16, tag="cn", name=f"cn_{b}_{n}_{h}")
            nc.vector.tensor_mul(cn, psc[h][0:DH, :], bc)
            nc.sync.dma_start(out=a2a_in[NS * b + n, DH * h:DH * (h + 1), :],
                              in_=cn)

    if True:
        # ---- pipelined: attention strip (b,n) right after its last chunk ----
        for c in range(BT // 512):
            emit_proj_chunk(c)
            emit_attn_strip(c // NS, c % NS)

        # ---- late constants (W_out, bias) — loaded during attention ----
        nc.sync.dma_start(out=wo_sb, in_=wo[:, :].rearrange("(a p) d -> p a d", p=128))
        bo_ap = bo[:]
        nc.gpsimd.dma_start(
            out=bo_bc,
            in_=bass.AP(tensor=bo_ap.tensor, offset=bo_ap.offset,
                        ap=[[0, 128]] + list(bo_ap.ap)),
        )

        # ---- exchange: head-sharded ctx -> token-sharded full ctx ----
        if cc:
            nc.gpsimd.collective_compute(
                "AllToAll",
                mybir.AluOpType.bypass,
                replica_groups=[list(range(NCORES))],
                ins=[a2a_in[:, :, :]],
                outs=[a2a_out[:, :, :]],
            )
        else:
            nc.sync.dma_start(out=a2a_out[:, :, :], in_=a2a_in[:, :, :])

        # ---- output projection for my 512-token slice ----
        ctxf_sb = opool.tile([128, NCORES, TLOC], bf16, bufs=1)
        nc.sync.dma_start(out=ctxf_sb, in_=a2a_out[:, :, :].rearrange("s p t -> p s t"))
        for m in range(TLOC // 128):
            y_sb = opool.tile([128, D], f32, tag="ysb", name=f"ysb_{m}")
            for st in range(D // 512):
                psy = psum.tile([128, 512], f32, tag="w512", bufs=2,
                                name=f"psy_{m}_{st}")
                for ft in range(NCORES):
                    nc.tensor.matmul(psy, ctxf_sb[:, ft, m * 128:(m + 1) * 128],
                                     wo_sb[:, ft, st * 512:(st + 1) * 512],
                                     start=(ft == 0), stop=(ft == NCORES - 1))
                nc.vector.tensor_add(y_sb[:, st * 512:(st + 1) * 512], psy,
                                     bo_bc[:, st * 512:(st + 1) * 512])
            nc.sync.dma_start(out=y_out[m * 128:(m + 1) * 128, :], in_=y_sb)


def _make_masks():
    # masks[r][p, f] = 1 where token (128*r + p) may be attended by token f
    p = np.arange(128)[:, None]
    f = np.arange(512)[None, :]
    return np.stack([(128 * r + p <= f) for r in range(4)]).astype(_BF)


_NC_CACHE = {}
LAST_RESULT = None


def _make_in_maps(input_emb, W_key, W_query, W_value, W_out, b_out):
    x = np.asarray(input_emb, dtype=np.float32).reshape(BT, D)
    wkf = np.asarray(W_key, dtype=np.float32)
    wqf = np.asarray(W_query, dtype=np.float32)
    wvf = np.asarray(W_value, dtype=np.float32)
    wof = np.asarray(W_out, dtype=np.float32)
    bof = np.ascontiguousarray(np.asarray(b_out, dtype=np.float32))

    x_bf = np.ascontiguousarray(x.astype(_BF))
    wo_bf = np.ascontiguousarray(wof.astype(_BF))
    masks = np.ascontiguousarray(_make_masks())

    in_maps = []
    for c in range(NCORES):
        sl = slice(HPC * DH * c, HPC * DH * (c + 1))
        in_maps.append({
            "x_bf": x_bf,
            "wk_loc": np.ascontiguousarray(wkf[:, sl].astype(_BF)),
            "wq_loc": np.ascontiguousarray(wqf[:, sl].astype(_BF)),
            "wv_loc": np.ascontiguousarray(wvf[:, sl].astype(_BF)),
            "w_out": wo_bf,
            "b_out": bof,
            "masks": masks,
        })
    return in_maps


def bench(inputs, n_iters=16, loop_n=1, cc=True):
    """Time the NEFF execution with device-resident inputs.

    Returns (per_call_seconds, y) where y is from the last iteration.
    """
    import time

    import jax
    from jax.sharding import Mesh, NamedSharding, PartitionSpec
    from jax.experimental.shard_map import shard_map

    import concourse.mybir as mybir
    from concourse import bass2jax

    bass2jax.install_neuronx_cc_hook()

    if ("nc", loop_n, cc) not in _NC_CACHE:
        _NC_CACHE[("nc", loop_n, cc)] = _build_nc(loop_n, cc)
    nc = _NC_CACHE[("nc", loop_n, cc)]
    in_maps = _make_in_maps(**inputs)

    partition_name = nc.partition_id_tensor.name if nc.partition_id_tensor else None
    in_names, out_names, out_avals, zero_outs = [], [], [], []
    for alloc in nc.m.functions[0].allocations:
        if not isinstance(alloc, mybir.MemoryLocationSet):
            continue
        name = alloc.memorylocations[0].name
        if alloc.kind == "ExternalInput":
            if name != partition_name:
                in_names.append(name)
        elif alloc.kind == "ExternalOutput":
            out_names.append(name)
            shape = tuple(alloc.tensor_shape)
            dtype = mybir.dt.np(alloc.dtype)
            out_avals.append(jax.core.ShapedArray(shape, dtype))
            zero_outs.append(np.zeros(shape, dtype))
    n_params = len(in_names)
    n_outs = len(out_avals)
    all_in_names = list(in_names) + list(out_names)
    if partition_name is not None:
        all_in_names.append(partition_name)
    donate = tuple(range(n_params, n_params + n_outs))

    def _body(*args):
        operands = list(args)
        if partition_name is not None:
            operands.append(bass2jax.partition_id_tensor())
        outs = bass2jax._bass_exec_p.bind(
            *operands,
            out_avals=tuple(out_avals),
            in_names=tuple(all_in_names),
            out_names=tuple(out_names),
            lowering_input_output_aliases=(),
            sim_require_finite=True,
            sim_require_nnan=True,
            nc=nc,
        )
        return tuple(outs)

    devices = jax.devices()[:NCORES]
    mesh = Mesh(np.asarray(devices), ("core",))
    in_specs = (PartitionSpec("core"),) * (n_params + n_outs)
    out_specs = (PartitionSpec("core"),) * len(out_names)
    fn = jax.jit(
        shard_map(_body, mesh=mesh, in_specs=in_specs, out_specs=out_specs,
                  check_rep=False),
        donate_argnums=donate, keep_unused=True,
    )
    sh = NamedSharding(mesh, PartitionSpec("core"))
    concat_in = [
        jax.device_put(
            np.concatenate([np.asarray(in_maps[c][nm]) for c in range(NCORES)],
                           axis=0), sh)
        for nm in in_names
    ]
    zero_sets = [
        [jax.device_put(np.zeros((NCORES * z.shape[0], *z.shape[1:]), z.dtype), sh)
         for z in zero_outs]
        for _ in range(n_iters + 1)
    ]
    out = fn(*concat_in, *zero_sets[0])
    jax.block_until_ready(out)
    t0 = time.perf_counter()
    outs = [fn(*concat_in, *zs) for zs in zero_sets[1:]]
    jax.block_until_ready(outs)
    t1 = time.perf_counter()
    per_call = (t1 - t0) / n_iters

    y = np.asarray(outs[-1][out_names.index("y_out")])
    y = y.reshape(NCORES, TLOC, D).reshape(BT, D)
    return per_call, np.ascontiguousarray(y.reshape(B, T, D).astype(np.float32))


def kernel(input_emb, W_key, W_query, W_value, W_out, b_out, _trace=False):
    from concourse.bass_utils import run_bass_kernel_spmd

    in_maps = _make_in_maps(input_emb, W_key, W_query, W_value, W_out, b_out)

    if ("nc", 1) not in _NC_CACHE:
        _NC_CACHE[("nc", 1)] = _build_nc(1)
    nc = _NC_CACHE[("nc", 1)]

    global LAST_RESULT
    LAST_RESULT = run_bass_kernel_spmd(nc, in_maps, core_ids=list(range(NCORES)),
                                       trace=_trace)
    y = np.concatenate([LAST_RESULT.results[c]["y_out"] for c in range(NCORES)],
                       axis=0)
    return np.ascontiguousarray(y.reshape(B, T, D).astype(np.float32))


def bench_hw(inputs, n1=1, n2=9, n_iters=12):
    """Two-point device-loop timing (collective replaced by local copy):
    returns (compute seconds/iter, cc-vs-copy delta seconds, y_from_n1)."""
    t_a, y = bench(inputs, n_iters=n_iters, loop_n=n1, cc=False)
    t_b, _ = bench(inputs, n_iters=n_iters, loop_n=n2, cc=False)
    t_cc, _ = bench(inputs, n_iters=max(n_iters, 24), loop_n=1, cc=True)
    t_nocc, _ = bench(inputs, n_iters=max(n_iters, 24), loop_n=1, cc=False)
    return (t_b - t_a) / (n2 - n1), t_cc - t_nocc, y
